# revision 1
# baseline (speedup 1.0000x reference)
"""MinGRU Trainium2 kernel (nn_MinGRUTriton_77309411812).

Reference computation (B=4, L=8192, D=1024, fp32):
    gates      = sigmoid(x @ Wg.T + bg)
    candidates = tanh   (x @ Wc.T + bc)
    h_t = gates_t * h_{t-1} + candidates_t        (h_0 = 0, scan along L)

Sharding (8 cores, no cross-core communication):
    core c -> batch b = c // 2, output-channel half eh = c % 2 (512 channels).

Host-side shard prep feeds each core transposed fp16 operands in
DMA-native layouts so the device kernel needs no transposes or casts and
every DMA descriptor is an 8 KB-contiguous per-partition run:
    xh[p, ci, kg, t] = x[b, ci*TC + t, kg*128 + p]   fp16 [128,16,8,512]
    wh[p, kg, e]     = W[eh*512 + e, kg*128 + p]     fp16 [128,8,512]
    (k = kg*128+p is the matmul contraction dim, on partitions)
fp16 operands run the PE at 1 cycle/row (4x faster than fp32) with a
fully-hidden 2-byte LDWEIGHTS and keep absmax relative error ~5e-4
(e5m10 rounding, fp32 PSUM accumulation; |x| < 6, |W| < 0.2, h ~ 5).

The matmul output lands as [e(partitions), t(free)], exactly the layout
tensor_tensor_scan needs (the scan runs along the free dim); h is stored
fp16 in hh[p, ci, eg, t] and un-permuted/upcast on the host.

Per 512-wide t-chunk: one 1 MB DMA load of the x slice (sync queue), 64
accumulating fp16 matmuls (PE), sigmoid/tanh straight out of PSUM with
fused per-partition bias (ACT), one tensor_tensor_scan per 128-channel
group (DVE, chained across chunks via initial=prev[:, -1:]), one 0.5 MB
DMA store.  Weight DMAs ride the scalar-engine HWDGE ring in parallel
with x chunk 0 on the sync ring (the two first-matmul gates); wc and the
x1/x2 prefetches are dep-deferred behind them so their bytes stay out of
the SDMA round-robin.  Dummy matmuls during the DMA wait hold the PE's
HAM clock gate at 2.4 GHz, and the last chunk runs as two 256-wide
halves to shorten the kernel-tail dependency chain.
"""

import sys

import numpy as np

try:
    import concourse.bass as bass  # noqa: F401
except ImportError:  # pragma: no cover - path fallback for fresh environments
    sys.path.insert(0, "/opt/trn_rl_repo")

import concourse.bass as bass
import concourse.mybir as mybir
import concourse.tile as tile
from concourse import bacc
from concourse.bass_utils import run_bass_kernel_spmd
from concourse.tile import add_dep_helper

B, L, D = 4, 8192, 1024
E = D // 2          # output channels per core
N_CORES = 8
TC = 512            # t-chunk (= matmul moving free dim = PSUM bank)
NK = D // 128       # contraction k-groups
NE = E // 128       # output-channel groups per core
NCH = L // TC       # t-chunks

F32 = mybir.dt.float32
F16 = mybir.dt.float16

_compiled = None


def _build():
    nc = bacc.Bacc("TRN2", target_bir_lowering=False, debug=False)

    xh = nc.dram_tensor("xh", [128, NCH, NK, TC], F16, kind="ExternalInput")
    wgh = nc.dram_tensor("wgh", [128, NE, NK, 128], F16, kind="ExternalInput")
    wch = nc.dram_tensor("wch", [128, NE, NK, 128], F16, kind="ExternalInput")
    bias = nc.dram_tensor("bias", [128, 2 * NE], F32, kind="ExternalInput")
    hh = nc.dram_tensor("hh", [128, NCH, NE, TC], F16, kind="ExternalOutput")

    with tile.TileContext(nc) as tc, \
            tc.tile_pool(name="wpool", bufs=1) as wpool, \
            tc.tile_pool(name="xpool", bufs=3) as xpool, \
            tc.tile_pool(name="gcpool", bufs=2) as gcpool, \
            tc.tile_pool(name="hpool", bufs=2) as hpool, \
            tc.tile_pool(name="pspool", bufs=6, space="PSUM") as pspool:

        b_all = wpool.tile([128, 2 * NE], F32)
        nc.sync.dma_start(out=b_all[:], in_=bias[:])
        bg_t = b_all[:, 0:NE]
        bc_t = b_all[:, NE:2 * NE]
        # Startup ordering: the first matmul gates on wg + x chunk 0 only.
        # wg rides the scalar HWDGE ring while x0 rides the sync ring (the
        # two rings drain in parallel); wc/x1/x2 are dep-deferred so their
        # bytes don't compete with the gating transfers.
        # wg arrives as 4 per-e-group pieces: the first matmul unit only
        # gates on piece 0 (256 KB) + x chunk 0, and later pieces stream in
        # behind the running PE.
        wg_t = wpool.tile([128, NE, NK, 128], F16)
        for eg in range(NE):
            i_wg = nc.scalar.dma_start(out=wg_t[:, eg], in_=wgh[:, eg])
        wc_t = wpool.tile([128, NE, NK, 128], F16)
        i_wc = nc.scalar.dma_start(out=wc_t[:], in_=wch[:])
        add_dep_helper(i_wc.ins, i_wg.ins, reason="defer wc behind wg")

        # Warm the PE's HAM clock gate (~3.4us of activity releases the
        # 1.2->2.4 GHz throttle) with dummy matmuls on a zeroed tile while
        # the startup DMAs are in flight.
        warm = wpool.tile([128, 512], F16)
        nc.vector.memset(warm[:], 0.0)
        warm_ps = pspool.tile([128, 512], F32, tag="warm", bufs=1)
        for _ in range(16):
            nc.tensor.matmul(warm_ps[:], warm[:, 0:128], warm[:, 0:512],
                             start=True, stop=True)

        h_prev = None
        for ci in range(NCH):
            x_t = xpool.tile([128, NK, TC], F16, tag="x")
            i_x = nc.sync.dma_start(out=x_t[:], in_=xh[:, ci])
            if ci == 1:
                add_dep_helper(i_x.ins, i_wg.ins, reason="defer x1 behind wg")
            elif ci == 2:
                add_dep_helper(i_x.ins, i_wc.ins, reason="defer x2 behind wc")

            g_t = gcpool.tile([128, NE, TC], F32, tag="g")
            c_t = gcpool.tile([128, NE, TC], F32, tag="c")
            h_t = hpool.tile([128, NE, TC], F16, tag="h")
            last = ci == NCH - 1

            def unit(w_t, b_t, out_t, func, eg, pieces):
                ps = pspool.tile([128, TC], F32, tag="ps", name="ps")
                for toff, tcw in pieces:
                    for kg in range(NK):
                        nc.tensor.matmul(
                            ps[:, toff:toff + tcw],
                            w_t[:, eg, kg, :],
                            x_t[:, kg, toff:toff + tcw],
                            start=(kg == 0),
                            stop=(kg == NK - 1),
                        )
                    nc.scalar.activation(
                        out_t[:, eg, toff:toff + tcw], ps[:, toff:toff + tcw],
                        func, bias=b_t[:, eg:eg + 1],
                    )

            SIG = mybir.ActivationFunctionType.Sigmoid
            TANH = mybir.ActivationFunctionType.Tanh
            whole = ((0, TC),)
            # The very last unit + scan of the kernel run as two 256-wide
            # halves so the final MM->ACT->scan->store chain is half as long.
            halved = ((0, TC // 2), (TC // 2, TC // 2))
            for eg in range(NE):
                unit(wg_t, bg_t, g_t, SIG, eg, whole)
            for eg in range(NE):
                unit(wc_t, bc_t, c_t, TANH, eg,
                     halved if last and eg == NE - 1 else whole)

            for eg in range(NE):
                pieces = halved if last and eg == NE - 1 else whole
                for toff, tcw in pieces:
                    if toff == 0:
                        init = 0.0 if ci == 0 else h_prev[:, eg, TC - 1:TC]
                    else:
                        init = h_t[:, eg, toff - 1:toff]
                    nc.vector.tensor_tensor_scan(
                        h_t[:, eg, toff:toff + tcw],
                        g_t[:, eg, toff:toff + tcw],
                        c_t[:, eg, toff:toff + tcw],
                        initial=init,
                        op0=mybir.AluOpType.mult,
                        op1=mybir.AluOpType.add,
                    )
                    if last:
                        # Per-group stores so the final store (the kernel-
                        # tail gate) only waits on the last scan piece.
                        nc.sync.dma_start(
                            out=hh[:, ci, eg, toff:toff + tcw],
                            in_=h_t[:, eg, toff:toff + tcw],
                        )
            if not last:
                nc.sync.dma_start(out=hh[:, ci], in_=h_t[:])
            h_prev = h_t

    nc.compile()
    return nc


def _get_compiled():
    global _compiled
    if _compiled is None:
        _compiled = _build()
    return _compiled


def make_in_maps(x, Wg, bg, Wc, bc):
    x = np.asarray(x, dtype=np.float32)
    # xh[p, ci, kg, t] = x[b, ci*TC + t, kg*128 + p]
    xhs = [
        np.ascontiguousarray(
            x[b].astype(np.float16)
            .reshape(NCH, TC, NK, 128)
            .transpose(3, 0, 2, 1)
        )
        for b in range(B)
    ]
    in_maps = []
    for c in range(N_CORES):
        b, eh = divmod(c, 2)
        sl = slice(eh * E, (eh + 1) * E)
        # wh[p, eg, kg, e'] = W[eh*512 + eg*128 + e', kg*128 + p]
        wgh = np.ascontiguousarray(
            np.asarray(Wg, np.float32)[sl].astype(np.float16)
            .reshape(NE, 128, NK, 128).transpose(3, 0, 2, 1))
        wch = np.ascontiguousarray(
            np.asarray(Wc, np.float32)[sl].astype(np.float16)
            .reshape(NE, 128, NK, 128).transpose(3, 0, 2, 1))
        in_maps.append({
            "xh": xhs[b],
            "wgh": wgh,
            "wch": wch,
            "bias": np.ascontiguousarray(np.stack(
                [np.asarray(bg, np.float32)[sl].reshape(NE, 128),
                 np.asarray(bc, np.float32)[sl].reshape(NE, 128)],
            ).reshape(2 * NE, 128).T),
        })
    return in_maps


def assemble_output(results):
    out = np.empty((B, L, D), np.float32)
    for c in range(N_CORES):
        b, eh = divmod(c, 2)
        hhv = results[c]["hh"]  # [128, NCH, NE, TC] fp16
        # out[b, ci*TC + t, eh*E + eg*128 + p] = hh[p, ci, eg, t]
        out[b, :, eh * E:(eh + 1) * E] = (
            hhv.transpose(1, 3, 2, 0).reshape(L, E).astype(np.float32))
    return out


def kernel(x, Wg, bg, Wc, bc, _trace=False, _trace_kwargs=None):
    nc = _get_compiled()
    in_maps = make_in_maps(x, Wg, bg, Wc, bc)
    res = run_bass_kernel_spmd(
        nc, in_maps, list(range(N_CORES)), trace=_trace,
        **(_trace_kwargs or {}),
    )
    out = assemble_output(res.results)
    if _trace:
        kernel.last_results = res
    return out

